# revision 31
# baseline (speedup 1.0000x reference)
"""Trainium2 Bass kernel for nn_EnvironmentSpecificDecoder.

Data-parallel over batch B=32 across 8 NeuronCores (NB=4 batches/core).

Matmuls in bf16 (FWL weight loads, half the input DMA bytes) except S23
which stays fp32r: its row-paired K=64 tiles stream two concurrent
half-partition matmuls (~0.25 ns/token-col), which beats serial bf16
full-K matmuls (~0.37), and bf16 row-paired hangs the HW.

Queue/engine split (each engine issuing DMAs gets its own HW queue, and
DMA_DIRECT2D costs ~600ns of issuing-engine time):
  sync (SP):     zin input stream (b0 in 2 chunks for fast start, b1-3
                 whole-batch prefetched at prior-batch start), b1-3
                 dispatched weights, per-oct-pair mu/sigma extraction
                 DMAs, final sg writes.
  scalar (Act):  prologue weight DMAs (reg, wpk, bc, b0 dispatch) so the
                 input stream never queues behind weights. In-loop ACTs.

Outputs are written in kernel-natural dense layouts (4KB runs) and
unpermuted on the host; softplus runs once at the end (exp and ln live
in different ACT tables, so interleaving them with the in-loop
relu/identity ACTs would thrash table loads).

Layout: pair pr=(qq,t01) holds (t, t+2) over tp; per-oct intermediates
are t-ascending. Per batch b, oct o (8 t's):
  stage1: 4 MMs  p1[(tp,l),(qq,t01,i)] = zz_pair^T @ A          (N=128)
  C1    : 4 MMs  pc[h2,(t01,i)] = Wc_pad^T zcT                  (N=256)
  S23   : per hh: 2 row-paired concurrent K=64 MMs (fp32r)    (N=512)
          ph[h,(tp,qq,t01,i)] = W1s^T zzt, W1s = W_sig@W1[env] fused on
          host (env dispatched per batch by regime via dynamic DMA)
  evac  : relu+bias fused PSUM->SBUF bf16 casts split Scalar/Vector
  S4+C2 : per quad 3 accumulating MMs (W2 halves + Wo, zero-padded to
          M=32) col-packed 2 quads per PSUM bank at partition bases 0/32
  out   : one [64,512] bias ACT per oct into a 2-oct staging tile; per
          oct-pair one DMA lifts mu rows {0,32} to DRAM and one lifts
          sigma rows {1,33} into a dense [16,512] block; per batch:
          softplus (exp+ln+add) + one dense sg DMA.
"""
import numpy as np
import ml_dtypes

N_CORES = 8
NB = 4          # batches per core
T = 64
D = 128
L = 64
H = 256
H2 = 128
NE = 8

_CACHE = {}


def _round_fp32r(x: np.ndarray) -> np.ndarray:
    """Round fp32 array to E8M11 (float32r) with round-to-nearest-even."""
    u = np.ascontiguousarray(x, dtype=np.float32).view(np.uint32)
    keep = np.uint32(12)
    half = np.uint32(1 << 11)
    lsb = (u >> keep) & np.uint32(1)
    return ((u + (half - np.uint32(1) + lsb)) >> keep << keep).view(np.float32)


def _build():
    import concourse.bacc as bacc
    import concourse.bass as bass
    import concourse.mybir as mybir
    from concourse.tile import TileContext

    F32 = mybir.dt.float32
    F32R = mybir.dt.float32r
    BF16 = mybir.dt.bfloat16
    AF = mybir.ActivationFunctionType
    ADD = mybir.AluOpType.add
    MAX = mybir.AluOpType.max
    ACT_E = mybir.EngineType.Activation
    SP_E = mybir.EngineType.SP

    nc = bacc.Bacc("TRN2", target_bir_lowering=False, debug=False)

    # inputs (host pre-packed, see _prepare_in_maps)
    zin_d = nc.dram_tensor("zin", [NB, 2, D, T * L], BF16, kind="ExternalInput")
    wpk_d = nc.dram_tensor("wpk", [D, 416], BF16, kind="ExternalInput")
    bc_d = nc.dram_tensor("bc", [H2, 1], F32, kind="ExternalInput")
    reg_d = nc.dram_tensor("reg", [1, NB], mybir.dt.int32, kind="ExternalInput")
    w1s_d = nc.dram_tensor("w1s", [NE, D, H], F32R, kind="ExternalInput")
    w2p_d = nc.dram_tensor("w2p", [NE, D, 2, 32], BF16, kind="ExternalInput")
    b12_d = nc.dram_tensor("b12", [NE, D, 3], F32, kind="ExternalInput")

    mu_d = nc.dram_tensor("mu", [NB, 2, 4096], F32, kind="ExternalOutput")
    sg_d = nc.dram_tensor("sg", [NB, 16, 512], F32, kind="ExternalOutput")

    with TileContext(nc) as tc:
        with (
            tc.tile_pool(name="const", bufs=1) as constp,
            tc.tile_pool(name="zin", bufs=2) as zinp,
            tc.tile_pool(name="ev", bufs=2) as evp,
            tc.tile_pool(name="stg", bufs=4) as stgp,
            tc.tile_pool(name="fin", bufs=1) as finp,
            tc.tile_pool(name="ps1", bufs=1, space="PSUM") as ps1,
            tc.tile_pool(name="ps23", bufs=1, space="PSUM") as ps23,
            tc.tile_pool(name="psc", bufs=2, space="PSUM") as psc,
            tc.tile_pool(name="ps4", bufs=1, space="PSUM") as ps4,
        ):
            # ---- scalar-queue prologue: reg + dispatch + static weights ----
            reg_sb = constp.tile([1, NB], mybir.dt.int32)
            nc.scalar.dma_start(reg_sb[:], reg_d[:])

            evals = []
            for b in range(NB):
                eng = ACT_E if b == 0 else SP_E
                evals.append(nc.values_load(
                    reg_sb[0:1, b : b + 1],
                    engines=[eng],
                    min_val=0, max_val=NE - 1,
                    skip_runtime_bounds_check=True,
                ))

            w1p_sb, w2_sb, b12_sb = [None] * NB, [None] * NB, [None] * NB

            def load_batch_weights(b):
                # per-batch dispatched weights (regime -> env); b0 on the
                # scalar queue (needed first), b1-3 on sync between inputs
                e = evals[b]
                de = nc.scalar if b == 0 else nc.sync
                w1p = constp.tile([D, H], F32R, name=f"w1p{b}", tag=f"w1p{b}")
                de.dma_start(
                    w1p[:], w1s_d[bass.ds(e, 1)].rearrange("o p h -> (o p) h"))
                w2 = constp.tile([D, 2, 32], BF16, name=f"w2{b}", tag=f"w2{b}")
                de.dma_start(
                    w2[:], w2p_d[bass.ds(e, 1)].rearrange("o p a k -> (o p) a k")
                )
                b12 = constp.tile([D, 3], F32, name=f"b12{b}", tag=f"b12{b}")
                de.dma_start(
                    b12[:], b12_d[bass.ds(e, 1)].rearrange("o p h -> (o p) h")
                )
                w1p_sb[b] = w1p
                w2_sb[b] = w2
                b12_sb[b] = b12

            zint = [None] * NB

            def load_batch_inputs(b, widths):
                cs = 0
                for cw in widths:
                    nc.sync.dma_start(
                        zint[b][:, :, cs : cs + cw],
                        zin_d[b, :, :, cs : cs + cw]
                        .rearrange("a p c -> p a c"),
                    )
                    cs += cw

            wpk_sb = constp.tile([D, 416], BF16)  # [ai | wc_pad | wo_pad]
            nc.scalar.dma_start(wpk_sb[:], wpk_d[:])
            bc_sb = constp.tile([H2, 1], F32)
            nc.scalar.dma_start(bc_sb[:], bc_d[:])
            ai_sb = wpk_sb[:, 0:128]
            wc_sb = wpk_sb[:, 128:384]
            wo_sb = wpk_sb[:, 384:416]
            load_batch_weights(0)

            zint[0] = zinp.tile([D, 2, T * L], BF16, name="zt0", tag="zt")
            load_batch_inputs(0, (1024, 3072))

            st_sig = finp.tile([NB * 16, 512], F32)
            ex_sig = finp.tile([NB * 16, 512], F32)
            bias_e = finp.tile([NB * 16, 1], F32)
            nc.vector.memset(bias_e[:], 0.01)
            bias_l = finp.tile([NB * 16, 1], F32)
            nc.vector.memset(bias_l[:], float(np.exp(0.01)))

            for b in range(NB):
                # prefetch next batch (weights then inputs) ahead of this
                # batch's staging DMAs on the sync queue; staging waits on
                # compute anyway, so the input stream keeps the lead
                if b + 1 < NB:
                    load_batch_weights(b + 1)
                    zint[b + 1] = zinp.tile([D, 2, T * L], BF16,
                                            name=f"zt{b+1}", tag="zt")
                    load_batch_inputs(b + 1, (T * L,))
                zz = zint[b][:, 0]
                zc = zint[b][:, 1]
                b1s = b12_sb[b][:, 0:2]
                b2b = b12_sb[b][:, 2:3]

                for o in range(8):
                    # ---- stage 1: 4 signal pair matmuls, N=128 ----
                    p1 = ps1.tile([D, 512], F32, tag="p1")
                    for qt in range(4):           # qt = qq*2+t01
                        pr = o * 4 + qt
                        nc.tensor.matmul(
                            p1[:, 128 * qt : 128 * (qt + 1)],
                            zz[:, 128 * pr : 128 * (pr + 1)],
                            ai_sb[:],
                            start=True, stop=True,
                        )
                    # ---- C1 (independent of zzt; hides the cast) ----
                    pcs = []
                    for qq in range(2):
                        pc = psc.tile([D, 512], F32, tag="pc")
                        for par in range(2):
                            nc.tensor.matmul(
                                pc[:, 256 * par : 256 * (par + 1)],
                                wc_sb[:, 128 * par : 128 * (par + 1)],
                                zc[:, 512 * o + 256 * qq :
                                   512 * o + 256 * qq + 256],
                                start=True, stop=True,
                            )
                        pcs.append(pc)

                    # ---- stage-1 evacuation: fp32 PSUM -> bf16 SBUF ----
                    zzt = evp.tile([D, 512], F32R, tag="zzt")
                    nc.vector.tensor_copy(zzt[:], p1[:])

                    # ---- S23 (hh-major) + h1 evac right after each hh ----
                    # h1 cols: hh*1024 + qq*512 + tq*128 + i  (tq = 2*tp+t01)
                    h1 = evp.tile([D, 2048], BF16, tag="h1")
                    h1v = h1[:].rearrange(
                        "p (hh qq par c) -> p hh qq par c", hh=2, qq=2, par=2)
                    for hh in range(2):
                        ph = ps23.tile([D, 1024], F32, tag=f"p23h{hh}")
                        for par in range(2):    # par = tp partition half
                            nc.tensor.matmul(
                                ph[:, 512 * par : 512 * par + 512],
                                w1p_sb[b][64 * par : 64 * par + 64,
                                          128 * hh : 128 * (hh + 1)],
                                zzt[64 * par : 64 * par + 64, :],
                                start=True, stop=True,
                            )
                        in_ap = ph[:].rearrange(
                            "p (par qq c) -> p par qq c", par=2, qq=2
                        ).transpose([0, 2, 1, 3])
                        out_ap = h1v[:, hh]
                        if hh == 0:
                            nc.scalar.activation(
                                out_ap, in_ap, AF.Relu,
                                bias=b1s[:, 0:1],
                            )
                        else:
                            nc.vector.tensor_scalar(
                                out_ap, in_ap,
                                b1s[:, 1:2], 0.0, ADD, MAX,
                            )

                    # ---- hc evac: relu(pc + bc) -> bf16 ----
                    hcs = evp.tile([D, 1024], BF16, tag="hcs")
                    nc.scalar.activation(
                        hcs[:, 0:512], pcs[0][:], AF.Relu, bias=bc_sb[:, 0:1])
                    nc.scalar.activation(
                        hcs[:, 512:768], pcs[1][:, 0:256], AF.Relu,
                        bias=bc_sb[:, 0:1])
                    nc.vector.tensor_scalar(
                        hcs[:, 768:1024], pcs[1][:, 256:512],
                        bc_sb[:, 0:1], 0.0, ADD, MAX)

                    # ---- S4 + C2: col-packed quads, 2 per PSUM bank ----
                    p4 = ps4.tile([D, 512], F32, tag="p4")
                    for qq in range(2):
                        bp = qq * 32
                        nc.tensor.matmul(
                            p4[bp : bp + 32, :], w2_sb[b][:, 0, :],
                            h1[:, 512 * qq : 512 * qq + 512],
                            start=True, stop=False,
                        )
                        nc.tensor.matmul(
                            p4[bp : bp + 32, :], wo_sb[:],
                            hcs[:, 512 * qq : 512 * qq + 512],
                            start=False, stop=False,
                        )
                        nc.tensor.matmul(
                            p4[bp : bp + 32, :], w2_sb[b][:, 1, :],
                            h1[:, 1024 + 512 * qq : 1024 + 512 * qq + 512],
                            start=False, stop=True,
                        )
                    # one bias pass covers both quads (rows 0,1,32,33)
                    if o % 2 == 0:
                        stb2 = stgp.tile([64, 1024], F32, tag="stb")
                    nc.scalar.activation(
                        stb2[:, 512 * (o % 2) : 512 * (o % 2) + 512],
                        p4[0:64, :], AF.Identity,
                        bias=b2b[0:64, 0:1])
                    if o % 2 == 1:
                        k = o // 2
                        for q in range(2):
                            # mu row {32q} -> dense DRAM (host unpermutes)
                            nc.sync.dma_start(
                                mu_d[b, q, 1024 * k : 1024 * k + 1024]
                                .rearrange("(w c) -> w c", w=2),
                                stb2[32 * q : 32 * q + 1, :],
                            )
                            # sigma row {32q+1} -> dense staging block
                            nc.sync.dma_start(
                                st_sig[16 * b + 8 * q + 2 * k :
                                       16 * b + 8 * q + 2 * k + 2, :],
                                stb2[32 * q + 1 : 32 * q + 2, :],
                            )
                # exp while the next batch computes (same ACT table as the
                # in-loop relu/identity evacs -> no table reload)
            # ---- sigma: softplus once; +0.01 folded exactly into the ACT
            # biases: ln(e^{x+0.01} + e^{0.01}) = ln(1+e^x) + 0.01 ----
            nc.scalar.activation(ex_sig[:], st_sig[:], AF.Exp, bias=bias_e[:])
            nc.scalar.activation(st_sig[:], ex_sig[:], AF.Ln, bias=bias_l[:])
            nc.sync.dma_start(
                sg_d[:].rearrange("b r c -> (b r) c"),
                st_sig[:],
            )

    nc.compile()
    return nc


def _get_nc():
    if "nc" not in _CACHE:
        _CACHE["nc"] = _build()
    return _CACHE["nc"]


def _prepare_in_maps(z_signal, z_corrupt, A, regime, W_sig, b_sig, W1e, b1e,
                     W2e, b2e, Wc, bc, Wo, bo):
    bf16 = ml_dtypes.bfloat16
    z_signal = np.asarray(z_signal, dtype=np.float32)
    z_corrupt = np.asarray(z_corrupt, dtype=np.float32)
    A = np.asarray(A, dtype=np.float32)
    regime = np.asarray(regime)
    W_sig = np.asarray(W_sig, dtype=np.float32)
    b_sig = np.asarray(b_sig, dtype=np.float32)
    W1e = np.asarray(W1e, dtype=np.float32)
    b1e = np.asarray(b1e, dtype=np.float32)
    W2e = np.asarray(W2e, dtype=np.float32)
    b2e = np.asarray(b2e, dtype=np.float32)
    Wc = np.asarray(Wc, dtype=np.float32)
    bc = np.asarray(bc, dtype=np.float32)
    Wo = np.asarray(Wo, dtype=np.float32)
    bo = np.asarray(bo, dtype=np.float32)

    eidx = np.where(regime >= NE, 0, regime).astype(np.int32)

    # ---- host weight transforms (env tables, replicated to all cores) ----
    wpk = np.zeros((D, 416), np.float32)
    wpk[:, 0:128] = A
    wpk[0:64, 128:256] = Wc            # [[Wc;0] | [0;Wc]]
    wpk[64:128, 256:384] = Wc
    wpk[:, 384:385] = Wo
    wpk = wpk.astype(bf16)
    w1s_half = np.einsum("lh,ehk->elk", W_sig, W1e)            # [E, L, H]
    w1s = _round_fp32r(
        np.ascontiguousarray(np.concatenate([w1s_half, w1s_half], axis=1)))
    b1s_full = np.einsum("h,ehk->ek", b_sig, W1e) + b1e        # [E, H]
    b12 = np.zeros((NE, D, 3), np.float32)
    b12[..., 0:2] = b1s_full.reshape(NE, 2, D).transpose(0, 2, 1)
    b12[:, 0::32, 2] = (b2e[:, 0] + bo[0])[:, None]
    b12[:, 1::32, 2] = b2e[:, 1][:, None]
    w2p = np.zeros((NE, D, 2, 32), np.float32)
    w2p[..., 0:2] = W2e.reshape(NE, 2, D, 2).transpose(0, 2, 1, 3)
    w2p = w2p.astype(bf16)
    bc_r = np.ascontiguousarray(bc[:, None])                   # [H2, 1]

    in_maps = []
    for c in range(N_CORES):
        b0 = c * NB
        zs4 = z_signal[b0 : b0 + NB]
        zc4 = z_corrupt[b0 : b0 + NB]
        # signal: [nb, D, (o,qq,t01,tp,l)] — pair pr=(o,qq,t01) holds (t,t+2)
        zt = zs4.transpose(0, 2, 1, 3).reshape(NB, D, 8, 2, 2, 2, L)
        zs_p = zt.transpose(0, 1, 2, 3, 5, 4, 6).reshape(NB, D, T * L)
        # corrupt (host-transposed): [nb, (tp,l), (o,qq,t01,i)]
        zcr = zc4.reshape(NB, 8, 2, 2, 2, D, L)
        zc_p = zcr.transpose(0, 3, 6, 1, 2, 4, 5).reshape(NB, D, T * L)
        zin = np.ascontiguousarray(
            np.stack([zs_p, zc_p], axis=1)).astype(bf16)       # [NB,2,D,TL]
        in_maps.append({
            "zin": zin,
            "wpk": wpk,
            "bc": bc_r,
            "reg": eidx[None, b0 : b0 + NB],
            "w1s": w1s,
            "w2p": w2p,
            "b12": b12,
        })
    return in_maps


def _unpermute(res):
    # mu_d[b, q, o*512 + tq*128 + d] ; sg_d[b, q*8+o, tq*128 + d]
    # t = o*8 + q*4 + tq
    mu = np.concatenate(
        [r["mu"].reshape(NB, 2, 8, 4, D).transpose(0, 2, 1, 3, 4)
         .reshape(NB, T, D) for r in res.results], axis=0)
    sg = np.concatenate(
        [r["sg"].reshape(NB, 2, 8, 4, D).transpose(0, 2, 1, 3, 4)
         .reshape(NB, T, D) for r in res.results], axis=0)
    return mu, sg


def kernel(z_signal, z_corrupt, A, regime, W_sig, b_sig, W1e, b1e, W2e, b2e,
           Wc, bc, Wo, bo):
    from concourse.bass_utils import run_bass_kernel_spmd

    in_maps = _prepare_in_maps(z_signal, z_corrupt, A, regime, W_sig, b_sig,
                               W1e, b1e, W2e, b2e, Wc, bc, Wo, bo)
    nc = _get_nc()
    res = run_bass_kernel_spmd(nc, in_maps, core_ids=list(range(N_CORES)))
    return _unpermute(res)


def run_traced(inputs_np):
    from concourse.bass_utils import run_bass_kernel_spmd

    in_maps = _prepare_in_maps(**inputs_np)
    nc = _get_nc()
    return run_bass_kernel_spmd(
        nc, in_maps, core_ids=list(range(N_CORES)), trace=True
    )
